# revision 17
# baseline (speedup 1.0000x reference)
# Transformer-XL style relative-position attention on 8 Trainium2 NeuronCores.
#
# Contract: kernel(**inputs) takes the FULL unsharded inputs and returns the
# FULL [8, 256, 1024] output. Internally shards data-parallel over batch:
# core b computes batch element b. No collectives needed.
#
# v4 design (vs the 114us v3):
#  * Software-pipelined main loop: iteration ft runs the score/softmax chain
#    of head pair ft-2 (STT band-add on DVE, exp on ACT, per-partition
#    normalize, one XBAR transpose per PAIR) while projecting k chunk ft,
#    BD pair ft and one v chunk.  The 25us of ACT exp work rides entirely
#    under the projection matmuls instead of forming its own phase.
#  * AV (val-stationary, PE column-group paired) + the incremental output
#    projection run as a dense PE-only tail -- everything they consume is
#    ready by then.
#  * wq is loaded in 4 column chunks and catt x-half first, so q-proj
#    starts as soon as ~1MB has landed instead of waiting for the full 9MB
#    load set (v3 idled the PE 12..28us).  wv/wo ride the SWDGE ring gated
#    behind the warmup's junk write so they don't steal early bandwidth.
#  * Per-pair band read and prob transpose are single DMAs ([128,2,2,384]
#    and [128,12,128]) to halve ring occupancy.
#  * Scratch rows are W=384 wide: [127 NEG pad | 257 BD values]; the right
#    pad of row i is the left pad of row i+1 under the shear read, and the
#    pad is written inline with the BD data (no separate NEG prefill).

import numpy as np

import concourse.bass as bass
import concourse.mybir as mybir
import concourse.tile as tile
from concourse import bacc, bass_utils
from concourse.tile import add_dep_helper
from contextlib import ExitStack

F32 = mybir.dt.float32
F16 = mybir.dt.float16
AF = mybir.ActivationFunctionType
OP = mybir.AluOpType

DIM = 1024
HEADS = 16
DHEAD = 64
B = 8
N = 256          # query tokens (x)
M = 256          # memory tokens (h)
T = M + N        # 512 keys
SCALE = DHEAD ** -0.5
NEG = -30000.0   # f16-representable; *0.125 still underflows exp
SW = 384         # scratch row width: 127 pad + 257 data
NS = 257         # valid relative offsets s = j - i in [0, 256]
PAD = 127        # left-pad columns per scratch row


def build_kernel():
    nc = bacc.Bacc("TRN2", target_bir_lowering=False, debug=False)

    catt_d = nc.dram_tensor("catT", [DIM, T], F16, kind="ExternalInput")
    wq_d = nc.dram_tensor("wq", [DIM, DIM], F16, kind="ExternalInput")
    wk_d = nc.dram_tensor("wk", [DIM, DIM], F16, kind="ExternalInput")
    wv_d = nc.dram_tensor("wv", [DIM, DIM], F16, kind="ExternalInput")
    wo_d = nc.dram_tensor("wo", [DIM, DIM], F16, kind="ExternalInput")
    rwst_d = nc.dram_tensor("rwst", [DIM, 258], F16, kind="ExternalInput")
    uuvv_d = nc.dram_tensor("uuvv", [128, 2], F32, kind="ExternalInput")
    out_d = nc.dram_tensor("out", [N, DIM], F16, kind="ExternalOutput")
    scr_d = nc.dram_tensor("scr", [HEADS, N, SW], F16)
    junk_d = nc.dram_tensor("warm_junk", [128, 512], F16)

    with tile.TileContext(nc) as tc, ExitStack() as ctx:
        _body(ctx, tc, catt_d, wq_d, wk_d, wv_d, wo_d, rwst_d, uuvv_d,
              out_d, scr_d, junk_d)

    nc.compile()
    return nc


def _body(ctx, tc, catt_d, wq_d, wk_d, wv_d, wo_d, rwst_d, uuvv_d, out_d,
          scr_d, junk_d):
    nc = tc.nc

    const = ctx.enter_context(tc.tile_pool(name="const", bufs=1))
    persist = ctx.enter_context(tc.tile_pool(name="persist", bufs=1))
    work = ctx.enter_context(tc.tile_pool(name="work", bufs=4))
    ps_m = ctx.enter_context(tc.tile_pool(name="ps_m", bufs=4, space="PSUM"))
    ps_a = ctx.enter_context(tc.tile_pool(name="ps_a", bufs=3, space="PSUM"))
    ps_v = ctx.enter_context(tc.tile_pool(name="ps_v", bufs=1, space="PSUM"))

    # ---------------- constants ----------------
    junk = const.tile([128, 512], F16, tag="junk", name="junk")
    nc.gpsimd.memset(junk, 1.0)
    uuvv = const.tile([128, 2], F32, tag="uuvv", name="uuvv_sb")

    # bsb: staging for BD scratch rows, [pad | data].  The pad columns are
    # memset once and never overwritten; per-head copies fill the data part.
    bsb = const.tile([128, 4, 2, SW], F16, tag="bsb", name="bsb")
    nc.gpsimd.memset(bsb, NEG)

    # ---------------- input loads ----------------
    catt_sb = persist.tile([128, 8, T], F16, tag="catt", name="catt_sb")
    wq_sb = persist.tile([128, 8, DIM], F16, tag="wq", name="wq_sb")
    wk_sb = persist.tile([128, 8, DIM], F16, tag="wk", name="wk_sb")
    wv_sb = persist.tile([128, 8, DIM], F16, tag="wv", name="wv_sb")
    wo_sb = persist.tile([128, 8, DIM], F16, tag="wo", name="wo_sb")
    rwst_sb = persist.tile([128, 8, 258], F16, tag="rwst", name="rwst_sb")

    def load_rows(eng, sb, dr, ncol, d0, d1, c0=0, c1=None):
        # sb[p, dt, c] = dr[dt*128 + p, c] for dt in [d0, d1), c in [c0, c1)
        if c1 is None:
            c1 = ncol
        src = bass.AP(dr[:, 0:1].tensor, d0 * 128 * ncol + c0,
                      [[ncol, 128], [128 * ncol, d1 - d0], [1, c1 - c0]])
        return eng.dma_start(out=sb[:, d0:d1, c0:c1], in_=src)

    # sync ring: q-proj inputs first (wq in 4 chunk-contiguous pieces -- the
    # host stores wq chunk-major -- then rwst); scalar ring: catt (stored
    # column-permuted [x|h] so each half is contiguous), wk, then wv/wo.
    # SWDGE ring keeps only the junk + band scratch writes.
    nc.sync.dma_start(out=uuvv, in_=uuvv_d[:, :])
    def load_catt(c0, dstc):
        src = bass.AP(catt_d[:, 0:1].tensor, c0,
                      [[T, 128], [128 * T, 8], [1, 256]])
        return nc.scalar.dma_start(out=catt_sb[:, :, dstc:dstc + 256], in_=src)
    load_catt(0, M)        # x tokens (stored first in DRAM)
    for c in range(4):
        # host stores wq as [4][1024][256] chunk-major; chunk c is contiguous
        src = bass.AP(wq_d[:, 0:1].tensor, c * 1024 * 256,
                      [[256, 128], [128 * 256, 8], [1, 256]])
        nc.sync.dma_start(out=wq_sb[:, :, c * 256:(c + 1) * 256], in_=src)
    load_rows(nc.sync, rwst_sb, rwst_d, 258, 0, 8)
    load_catt(256, 0)      # h tokens
    load_rows(nc.scalar, wk_sb, wk_d, DIM, 0, 8, 0, 512)
    load_rows(nc.scalar, wk_sb, wk_d, DIM, 0, 8, 512, 1024)
    load_rows(nc.scalar, wv_sb, wv_d, DIM, 0, 8)
    load_rows(nc.scalar, wo_sb, wo_d, DIM, 0, 8)

    # ---------------- PE warm-up (p-state ramp + covers load latency) ------
    pwarm = ps_m.tile([128, 512], F32, tag="pm", name="ps_warm")
    for wi in range(10):
        nc.tensor.matmul(pwarm, junk[:, 0:128], junk,
                         start=(wi == 0), stop=(wi == 9))
    junk2 = const.tile([128, 512], F16, tag="junk2", name="junk2")
    nc.vector.tensor_copy(junk2, pwarm)
    nc.gpsimd.dma_start(out=junk_d[:, :], in_=junk2)

    # ---------------- q projection (x tokens only) ----------------
    # quT/qvT[p, ft, i]: feature-major q with u/v folded in.
    quT = persist.tile([128, 8, N], F16, tag="quT", name="quT")
    qvT = persist.tile([128, 8, N], F16, tag="qvT", name="qvT")
    for ft in range(8):
        pq = ps_m.tile([128, 512], F32, tag="pm", name=f"ps_q{ft}")
        for dt in range(8):
            nc.tensor.matmul(pq[:, 0:N], wq_sb[:, dt, ft * 128:(ft + 1) * 128],
                             catt_sb[:, dt, M:T], start=(dt == 0), stop=(dt == 7))
        nc.vector.tensor_scalar_add(quT[:, ft, :], pq[:, 0:N], uuvv[:, 0:1])
        nc.vector.tensor_scalar_add(qvT[:, ft, :], pq[:, 0:N], uuvv[:, 1:2])

    # ---------------- pipelined main loop ----------------
    # kT[p, ft, j]: feature-major keys.   val[p, jt, h, d]: token-major vals.
    # BD[i, s] = (q+v)[i] . RW[s] staged into bsb rows [127 NEG | 257 data],
    # written whole-row contiguous, read back sheared per PAIR.
    kT = persist.tile([128, 8, T], F16, tag="kT", name="kT")
    val = persist.tile([128, 4, HEADS, DHEAD], F16, tag="val", name="val")
    bandp = [None] * 8   # per-pair sheared band tiles
    attTp = [None] * 8   # per-pair transposed probability tiles

    def bd_pair(ft):
        # heads 2ft (rows 0:64) and 2ft+1 (rows 64:128): alternate PE row
        # groups so the K=64 matmuls overlap.
        pbs = {}
        for qb in range(2):
            for hp in range(2):
                hh, ro = 2 * ft + hp, hp * 64
                pb = ps_m.tile([128, 512], F32, tag="pm",
                               name=f"ps_b{hh}_{qb}")
                nc.tensor.matmul(pb[:, 0:NS],
                                 qvT[ro:ro + 64, ft, qb * 128:(qb + 1) * 128],
                                 rwst_sb[ro:ro + 64, ft, 0:NS],
                                 start=True, stop=True)
                pbs[(hp, qb)] = pb
        ws = []
        for hp in range(2):
            hh = 2 * ft + hp
            slot = hh % 4
            for qb in range(2):
                # split psum->sbuf copies between ACT and DVE
                if hp == 0:
                    nc.scalar.copy(bsb[:, slot, qb, PAD:SW],
                                   pbs[(hp, qb)][:, 0:NS])
                else:
                    nc.vector.tensor_copy(bsb[:, slot, qb, PAD:SW],
                                          pbs[(hp, qb)][:, 0:NS])
            dst = bass.AP(scr_d[0][:, 0:1].tensor, hh * N * SW,
                          [[SW, 128], [128 * SW, 2], [1, SW]])
            ws.append(nc.gpsimd.dma_start(out=dst, in_=bsb[:, slot, :, :]))
        # read on the SWDGE ring: it sits right behind the writes it waits
        # on, so the RAW wait doesn't head-of-line-block the transposes
        # (43us of sync-ring stall when this was on nc.sync).
        band = work.tile([128, 2, 2, SW], F16, tag="band", name=f"band{ft}",
                         bufs=4)
        src = bass.AP(scr_d[0][:, 0:1].tensor, 2 * ft * N * SW + PAD,
                      [[SW - 1, 128], [N * SW, 2], [128 * SW, 2], [1, SW]])
        r = nc.gpsimd.dma_start(out=band[:, :, :, :], in_=src)
        add_dep_helper(r.ins, ws[0].ins, sync=True, reason="scratch RAW")
        add_dep_helper(r.ins, ws[1].ins, sync=True, reason="scratch RAW")
        bandp[ft] = band

    def scores_pair(ft):
        band = bandp[ft]
        att = work.tile([128, 2, 2, SW], F16, tag="att", name=f"att{ft}",
                        bufs=4)
        # 4 score matmuls interleaved (h.q0, h1.q0, h.q1, h1.q1): PE row
        # groups 0/64 alternate and overlap.
        pas = {}
        for qb in range(2):
            for hp in range(2):
                hh, ro = 2 * ft + hp, hp * 64
                pa = ps_a.tile([128, SW], F32, tag="pa", name=f"ps_a{hh}_{qb}")
                nc.tensor.matmul(pa,
                                 quT[ro:ro + 64, ft, qb * 128:(qb + 1) * 128],
                                 kT[ro:ro + 64, ft, qb * 128:qb * 128 + SW],
                                 start=True, stop=True)
                pas[(hp, qb)] = pa
        # Emit all 4 band-adds first, then exps, then rcp+normalize: the DVE
        # queue is strict FIFO, and an rcp right after its STT would stall
        # the queue waiting for the ACT exp between them.
        attps, rcps, ssums = {}, {}, {}
        for hp in range(2):
            hh = 2 * ft + hp
            attps[hp] = work.tile([128, 2, SW], F16, tag="attp",
                                  name=f"attp{hh}", bufs=4)
            rcps[hp] = work.tile([128, 2], F32, tag="rcp", name=f"rcp{hh}",
                                 bufs=4)
            for qb in range(2):
                nc.vector.scalar_tensor_tensor(attps[hp][:, qb, :],
                                               pas[(hp, qb)], 1.0,
                                               band[:, hp, qb, :],
                                               OP.mult, OP.add)
        for hp in range(2):
            hh = 2 * ft + hp
            for qb in range(2):
                ssum = work.tile([128, 1], F32, tag="ssum",
                                 name=f"ss{hh}_{qb}", bufs=12)
                nc.scalar.activation(att[:, hp, qb, :], attps[hp][:, qb, :],
                                     AF.Exp, bias=0.0, scale=SCALE,
                                     accum_out=ssum)
                ssums[(hp, qb)] = ssum
        for hp in range(2):
            for qb in range(2):
                # normalize in place on DVE (ACT-Copy costs 630ns here and
                # made ACT the loop pacer; gpsimd takes 5.7us -- measured)
                nc.vector.reciprocal(rcps[hp][:, qb:qb + 1], ssums[(hp, qb)])
                nc.vector.tensor_scalar_mul(att[:, hp, qb, :],
                                            att[:, hp, qb, :],
                                            rcps[hp][:, qb:qb + 1])
        # attT[j', k, i'] = att[i', k // 3, (k % 3) * 128 + j'], k in [0,12):
        # one XBAR transpose per pair; k = hp*6 + qb*3 + w.
        attT = work.tile([128, 12, 128], F16, tag="attT", name=f"attT{ft}",
                         bufs=8)
        nc.sync.dma_start(out=attT[:, :, :], in_=att[:, 0:2, 0:2, :],
                          transpose=True)
        attTp[ft] = attT

    for ft in range(8):
        if ft >= 2:
            scores_pair(ft - 2)
        pk = ps_m.tile([128, 512], F32, tag="pm", name=f"ps_k{ft}")
        for dt in range(8):
            nc.tensor.matmul(pk, wk_sb[:, dt, ft * 128:(ft + 1) * 128],
                             catt_sb[:, dt, :], start=(dt == 0), stop=(dt == 7))
        nc.scalar.copy(kT[:, ft, :], pk)
        bd_pair(ft)
        # one v chunk per iteration: nh0 chunks first so AV pairs 0-3 are
        # unblocked right after the loop.
        jt, nh = ft % 4, ft // 4
        pv = ps_m.tile([128, 512], F32, tag="pm", name=f"ps_v{jt}_{nh}")
        for dt in range(8):
            nc.tensor.matmul(pv, catt_sb[:, dt, jt * 128:(jt + 1) * 128],
                             wv_sb[:, dt, nh * 512:(nh + 1) * 512],
                             start=(dt == 0), stop=(dt == 7))
        nc.vector.tensor_copy(val[:, jt, nh * 8:(nh + 1) * 8, :], pv)
    scores_pair(6)
    scores_pair(7)

    # ---------------- AV + incremental output projection (PE tail) --------
    # AV is val-stationary: pav[d, i] = val[j, d].T @ attT[j, i], with head
    # pairs on PE column groups 0/64.  Output is feature-major, exactly the
    # lhsT the output projection needs, so chunk ft accumulates into the 4
    # held po banks right after its pair completes.
    aoT = [persist.tile([128, 8, 128], F16, tag=f"aoT{qb}", name=f"aoT{qb}")
           for qb in range(2)]
    pav = ps_v.tile([128, 2, 128], F32, tag="pav", name="pav")
    po = [[None, None], [None, None]]
    osb = persist.tile([128, 2, DIM], F16, tag="osb", name="osb")

    for ft in range(8):
        attT = attTp[ft]
        for qb in range(2):
            for hp in range(2):
                hh = 2 * ft + hp
                for w in range(3):
                    nc.tensor.matmul(pav[hp * 64:hp * 64 + 64, qb, :],
                                     val[:, qb + w, hh, :],
                                     attT[:, hp * 6 + qb * 3 + w, :],
                                     start=(w == 0), stop=(w == 2))
        for qb in range(2):
            nc.vector.tensor_copy(aoT[qb][:, ft, :], pav[:, qb, :])
        for qb in range(2):
            for nh in range(2):
                if ft == 0:
                    po[qb][nh] = ps_m.tile([128, 512], F32, tag="pm",
                                           name=f"ps_o{qb}_{nh}")
                nc.tensor.matmul(po[qb][nh], aoT[qb][:, ft, :],
                                 wo_sb[:, ft, nh * 512:(nh + 1) * 512],
                                 start=(ft == 0), stop=(ft == 7))

    # ---------------- output writeback ----------------
    for qb in range(2):
        nc.scalar.copy(osb[:, qb, 0:512], po[qb][0])
        nc.vector.tensor_copy(osb[:, qb, 512:1024], po[qb][1])
    dst = bass.AP(out_d[:, 0:1].tensor, 0, [[DIM, 128], [128 * DIM, 2], [1, DIM]])
    nc.sync.dma_start(out=dst, in_=osb[:, :, :])


def host_prep(inputs):
    x = np.asarray(inputs["x"], dtype=np.float32)
    h = np.asarray(inputs["h"], dtype=np.float32)
    wqkv = np.asarray(inputs["Wqkv"], dtype=np.float32)
    wkr = np.asarray(inputs["Wkr"], dtype=np.float32)
    r = np.asarray(inputs["R"], dtype=np.float32)
    u = np.asarray(inputs["u"], dtype=np.float32)
    v = np.asarray(inputs["v"], dtype=np.float32)
    wout = np.asarray(inputs["Wout"], dtype=np.float32)

    # wq is stored chunk-major ([4][1024][256] flattened) so each 2-ft
    # column chunk is one contiguous 512KB block for the chunked load.
    wq_f = wqkv[:, 0:DIM].astype(np.float16)
    wq = np.ascontiguousarray(
        np.stack([wq_f[:, c * 256:(c + 1) * 256] for c in range(4)], 0)
    ).reshape(DIM, DIM)
    wk = np.ascontiguousarray(wqkv[:, DIM:2 * DIM].astype(np.float16))
    wv = np.ascontiguousarray(wqkv[:, 2 * DIM:3 * DIM].astype(np.float16))
    wo = np.ascontiguousarray(wout.astype(np.float16))

    # positional keys: only offsets s = j - i in [0, 256] are unmasked;
    # RW row for offset s is (R @ Wkr)[(s + 768) % 1024].
    rows = (np.arange(NS) + 768) % 1024
    rws = r[rows] @ wkr                        # [257, 1024] f32
    rwst = np.zeros((DIM, 258), dtype=np.float16)
    rwst[:, 0:NS] = rws.T.astype(np.float16)

    uuvv = np.stack([np.tile(u, 2), np.tile(v, 2)], axis=1)
    uuvv = np.ascontiguousarray(uuvv.astype(np.float32))

    catts = []
    for b in range(B):
        cat = np.concatenate([h[b], x[b]], axis=0)          # [512, 1024]
        catT = cat.T.astype(np.float16)                     # [dim, h|x]
        # stored column-permuted [x|h] so each half loads contiguously
        catts.append(np.ascontiguousarray(
            np.concatenate([catT[:, M:T], catT[:, 0:M]], axis=1)))

    shared = {"wq": wq, "wk": wk, "wv": wv, "wo": wo, "rwst": rwst,
              "uuvv": uuvv}
    return catts, shared


_NC_CACHE = {}


def _get_nc():
    if "nc" not in _NC_CACHE:
        _NC_CACHE["nc"] = build_kernel()
    return _NC_CACHE["nc"]


def _run(inputs, trace=False):
    catts, shared = host_prep(inputs)
    nc = _get_nc()
    in_maps = [dict(shared, catT=catts[b]) for b in range(B)]
    res = bass_utils.run_bass_kernel_spmd(
        nc, in_maps, core_ids=list(range(B)), trace=trace)
    out = np.stack([res.results[b]["out"].astype(np.float32)
                    for b in range(B)])
    return out, res


def kernel(**inputs):
    out, _ = _run(inputs, trace=False)
    return out


# revision 21
# speedup vs baseline: 1.0607x; 1.0607x over previous
# Transformer-XL style relative-position attention on 8 Trainium2 NeuronCores.
#
# Contract: kernel(**inputs) takes the FULL unsharded inputs and returns the
# FULL [8, 256, 1024] output. Internally shards data-parallel over batch:
# core b computes batch element b. No collectives needed.
#
# v4 design (vs the 114us v3):
#  * Software-pipelined main loop: iteration ft runs the score/softmax chain
#    of head pair ft-2 (STT band-add on DVE, exp on ACT, per-partition
#    normalize, one XBAR transpose per PAIR) while projecting k chunk ft,
#    BD pair ft and one v chunk.  The 25us of ACT exp work rides entirely
#    under the projection matmuls instead of forming its own phase.
#  * AV (val-stationary, PE column-group paired) + the incremental output
#    projection run as a dense PE-only tail -- everything they consume is
#    ready by then.
#  * wq is loaded in 4 column chunks and catt x-half first, so q-proj
#    starts as soon as ~1MB has landed instead of waiting for the full 9MB
#    load set (v3 idled the PE 12..28us).  wv/wo ride the SWDGE ring gated
#    behind the warmup's junk write so they don't steal early bandwidth.
#  * Per-pair band read and prob transpose are single DMAs ([128,2,2,384]
#    and [128,12,128]) to halve ring occupancy.
#  * Scratch rows are W=384 wide: [127 NEG pad | 257 BD values]; the right
#    pad of row i is the left pad of row i+1 under the shear read, and the
#    pad is written inline with the BD data (no separate NEG prefill).

import numpy as np

import concourse.bass as bass
import concourse.mybir as mybir
import concourse.tile as tile
from concourse import bacc, bass_utils
from concourse.tile import add_dep_helper
from contextlib import ExitStack

F32 = mybir.dt.float32
F16 = mybir.dt.float16
AF = mybir.ActivationFunctionType
OP = mybir.AluOpType

DIM = 1024
HEADS = 16
DHEAD = 64
B = 8
N = 256          # query tokens (x)
M = 256          # memory tokens (h)
T = M + N        # 512 keys
SCALE = DHEAD ** -0.5
NEG = -30000.0   # f16-representable; *0.125 still underflows exp
SW = 384         # scratch row width: 127 pad + 257 data
NS = 257         # valid relative offsets s = j - i in [0, 256]
PAD = 127        # left-pad columns per scratch row


def build_kernel():
    nc = bacc.Bacc("TRN2", target_bir_lowering=False, debug=False)

    catt_d = nc.dram_tensor("catT", [DIM, T], F16, kind="ExternalInput")
    wq_d = nc.dram_tensor("wq", [DIM, DIM], F16, kind="ExternalInput")
    wk_d = nc.dram_tensor("wk", [DIM, DIM], F16, kind="ExternalInput")
    wv_d = nc.dram_tensor("wv", [DIM, DIM], F16, kind="ExternalInput")
    wo_d = nc.dram_tensor("wo", [DIM, DIM], F16, kind="ExternalInput")
    rwst_d = nc.dram_tensor("rwst", [DIM, 258], F16, kind="ExternalInput")
    uuvv_d = nc.dram_tensor("uuvv", [128, 2], F32, kind="ExternalInput")
    out_d = nc.dram_tensor("out", [N, DIM], F16, kind="ExternalOutput")
    scr_d = nc.dram_tensor("scr", [HEADS, N, SW], F16)
    junk_d = nc.dram_tensor("warm_junk", [128, 512], F16)

    with tile.TileContext(nc) as tc, ExitStack() as ctx:
        _body(ctx, tc, catt_d, wq_d, wk_d, wv_d, wo_d, rwst_d, uuvv_d,
              out_d, scr_d, junk_d)

    nc.compile()
    return nc


def _body(ctx, tc, catt_d, wq_d, wk_d, wv_d, wo_d, rwst_d, uuvv_d, out_d,
          scr_d, junk_d):
    nc = tc.nc

    const = ctx.enter_context(tc.tile_pool(name="const", bufs=1))
    persist = ctx.enter_context(tc.tile_pool(name="persist", bufs=1))
    work = ctx.enter_context(tc.tile_pool(name="work", bufs=4))
    ps_m = ctx.enter_context(tc.tile_pool(name="ps_m", bufs=4, space="PSUM"))
    ps_a = ctx.enter_context(tc.tile_pool(name="ps_a", bufs=3, space="PSUM"))
    ps_v = ctx.enter_context(tc.tile_pool(name="ps_v", bufs=1, space="PSUM"))

    # ---------------- constants ----------------
    junk = const.tile([128, 512], F16, tag="junk", name="junk")
    nc.gpsimd.memset(junk, 1.0)
    uuvv = const.tile([128, 2], F32, tag="uuvv", name="uuvv_sb")

    # bsb: staging for BD scratch rows, [pad | data].  The pad columns are
    # memset once and never overwritten; per-head copies fill the data part.
    bsb = const.tile([128, 4, 2, SW], F16, tag="bsb", name="bsb")
    nc.gpsimd.memset(bsb, NEG)

    # ---------------- input loads ----------------
    catt_sb = persist.tile([128, 8, T], F16, tag="catt", name="catt_sb")
    wq_sb = persist.tile([128, 8, DIM], F16, tag="wq", name="wq_sb")
    wk_sb = persist.tile([128, 8, DIM], F16, tag="wk", name="wk_sb")
    wv_sb = persist.tile([128, 8, DIM], F16, tag="wv", name="wv_sb")
    wo_sb = persist.tile([128, 8, DIM], F16, tag="wo", name="wo_sb")
    rwst_sb = persist.tile([128, 8, 258], F16, tag="rwst", name="rwst_sb")

    def load_rows(eng, sb, dr, ncol, d0, d1, c0=0, c1=None):
        # sb[p, dt, c] = dr[dt*128 + p, c] for dt in [d0, d1), c in [c0, c1)
        if c1 is None:
            c1 = ncol
        src = bass.AP(dr[:, 0:1].tensor, d0 * 128 * ncol + c0,
                      [[ncol, 128], [128 * ncol, d1 - d0], [1, c1 - c0]])
        return eng.dma_start(out=sb[:, d0:d1, c0:c1], in_=src)

    # sync ring: q-proj inputs first (wq in 4 chunk-contiguous pieces -- the
    # host stores wq chunk-major -- then rwst); scalar ring: catt (stored
    # column-permuted [x|h] so each half is contiguous), wk, then wv/wo.
    # SWDGE ring keeps only the junk + band scratch writes.
    nc.sync.dma_start(out=uuvv, in_=uuvv_d[:, :])
    def load_catt(c0, dstc):
        src = bass.AP(catt_d[:, 0:1].tensor, c0,
                      [[T, 128], [128 * T, 8], [1, 256]])
        return nc.scalar.dma_start(out=catt_sb[:, :, dstc:dstc + 256], in_=src)
    load_catt(0, M)        # x tokens (stored first in DRAM)
    for c in range(4):
        # host stores wq as [4][1024][256] chunk-major; chunk c is contiguous
        src = bass.AP(wq_d[:, 0:1].tensor, c * 1024 * 256,
                      [[256, 128], [128 * 256, 8], [1, 256]])
        nc.sync.dma_start(out=wq_sb[:, :, c * 256:(c + 1) * 256], in_=src)
    load_rows(nc.sync, rwst_sb, rwst_d, 258, 0, 8)
    load_catt(256, 0)      # h tokens
    load_rows(nc.scalar, wk_sb, wk_d, DIM, 0, 8, 0, 512)
    load_rows(nc.scalar, wk_sb, wk_d, DIM, 0, 8, 512, 1024)
    load_rows(nc.scalar, wv_sb, wv_d, DIM, 0, 8)
    load_rows(nc.scalar, wo_sb, wo_d, DIM, 0, 8)

    # ---------------- PE warm-up (p-state ramp + covers load latency) ------
    pwarm = ps_m.tile([128, 512], F32, tag="pm", name="ps_warm")
    for wi in range(10):
        nc.tensor.matmul(pwarm, junk[:, 0:128], junk,
                         start=(wi == 0), stop=(wi == 9))
    junk2 = const.tile([128, 512], F16, tag="junk2", name="junk2")
    nc.vector.tensor_copy(junk2, pwarm)
    nc.gpsimd.dma_start(out=junk_d[:, :], in_=junk2)

    # ---------------- q projection (x tokens only) ----------------
    # quT/qvT[p, ft, i]: feature-major q with u/v folded in.
    quT = persist.tile([128, 8, N], F16, tag="quT", name="quT")
    qvT = persist.tile([128, 8, N], F16, tag="qvT", name="qvT")
    for ft in range(8):
        pq = ps_m.tile([128, 512], F32, tag="pm", name=f"ps_q{ft}")
        for dt in range(8):
            nc.tensor.matmul(pq[:, 0:N], wq_sb[:, dt, ft * 128:(ft + 1) * 128],
                             catt_sb[:, dt, M:T], start=(dt == 0), stop=(dt == 7))
        nc.vector.tensor_scalar_add(quT[:, ft, :], pq[:, 0:N], uuvv[:, 0:1])
        nc.vector.tensor_scalar_add(qvT[:, ft, :], pq[:, 0:N], uuvv[:, 1:2])

    # ---------------- pipelined main loop ----------------
    # kT[p, ft, j]: feature-major keys.   val[p, jt, h, d]: token-major vals.
    # BD[i, s] = (q+v)[i] . RW[s] staged into bsb rows [127 NEG | 257 data],
    # written whole-row contiguous, read back sheared per PAIR.
    kT = persist.tile([128, 8, T], F16, tag="kT", name="kT")
    val = persist.tile([128, 4, HEADS, DHEAD], F16, tag="val", name="val")
    bandp = [None] * 8   # per-pair sheared band tiles
    attTp = [None] * 8   # per-pair transposed probability tiles
    wr_insts = [None] * 8  # per-pair scratch-write instructions

    def bd_pair(ft):
        # heads 2ft (rows 0:64) and 2ft+1 (rows 64:128): alternate PE row
        # groups so the K=64 matmuls overlap.
        pbs = {}
        for qb in range(2):
            for hp in range(2):
                hh, ro = 2 * ft + hp, hp * 64
                pb = ps_m.tile([128, 512], F32, tag="pm",
                               name=f"ps_b{hh}_{qb}")
                nc.tensor.matmul(pb[:, 0:NS],
                                 qvT[ro:ro + 64, ft, qb * 128:(qb + 1) * 128],
                                 rwst_sb[ro:ro + 64, ft, 0:NS],
                                 start=True, stop=True)
                pbs[(hp, qb)] = pb
        ws = []
        for hp in range(2):
            hh = 2 * ft + hp
            slot = hh % 4
            for qb in range(2):
                # split psum->sbuf copies between ACT and DVE
                if hp == 0:
                    nc.scalar.copy(bsb[:, slot, qb, PAD:SW],
                                   pbs[(hp, qb)][:, 0:NS])
                else:
                    nc.vector.tensor_copy(bsb[:, slot, qb, PAD:SW],
                                          pbs[(hp, qb)][:, 0:NS])
            dst = bass.AP(scr_d[0][:, 0:1].tensor, hh * N * SW,
                          [[SW, 128], [128 * SW, 2], [1, SW]])
            ws.append(nc.gpsimd.dma_start(out=dst, in_=bsb[:, slot, :, :]))
        wr_insts[ft] = ws

    def band_read(ft):
        # issued two iterations after the writes, so the RAW wait is already
        # satisfied when the sync ring reaches it (issuing it eagerly parked
        # the ring head-of-line for 43us and starved the transposes).
        band = work.tile([128, 2, 2, SW], F16, tag="band", name=f"band{ft}",
                         bufs=4)
        src = bass.AP(scr_d[0][:, 0:1].tensor, 2 * ft * N * SW + PAD,
                      [[SW - 1, 128], [N * SW, 2], [128 * SW, 2], [1, SW]])
        r = nc.sync.dma_start(out=band[:, :, :, :], in_=src)
        for w in wr_insts[ft]:
            add_dep_helper(r.ins, w.ins, sync=True, reason="scratch RAW")
        bandp[ft] = band

    def scores_pair(ft):
        band = bandp[ft]
        att = work.tile([128, 2, 2, SW], F16, tag="att", name=f"att{ft}",
                        bufs=4)
        # 4 score matmuls interleaved (h.q0, h1.q0, h.q1, h1.q1): PE row
        # groups 0/64 alternate and overlap.
        pas = {}
        for qb in range(2):
            for hp in range(2):
                hh, ro = 2 * ft + hp, hp * 64
                pa = ps_a.tile([128, SW], F32, tag="pa", name=f"ps_a{hh}_{qb}")
                nc.tensor.matmul(pa,
                                 quT[ro:ro + 64, ft, qb * 128:(qb + 1) * 128],
                                 kT[ro:ro + 64, ft, qb * 128:qb * 128 + SW],
                                 start=True, stop=True)
                pas[(hp, qb)] = pa
        # Emit all 4 band-adds first, then exps, then rcp+normalize: the DVE
        # queue is strict FIFO, and an rcp right after its STT would stall
        # the queue waiting for the ACT exp between them.
        attps, rcps, ssums = {}, {}, {}
        for hp in range(2):
            hh = 2 * ft + hp
            attps[hp] = work.tile([128, 2, SW], F16, tag="attp",
                                  name=f"attp{hh}", bufs=4)
            rcps[hp] = work.tile([128, 2], F32, tag="rcp", name=f"rcp{hh}",
                                 bufs=4)
            for qb in range(2):
                nc.vector.scalar_tensor_tensor(attps[hp][:, qb, :],
                                               pas[(hp, qb)], 1.0,
                                               band[:, hp, qb, :],
                                               OP.mult, OP.add)
        for hp in range(2):
            hh = 2 * ft + hp
            for qb in range(2):
                ssum = work.tile([128, 1], F32, tag="ssum",
                                 name=f"ss{hh}_{qb}", bufs=12)
                nc.scalar.activation(att[:, hp, qb, :], attps[hp][:, qb, :],
                                     AF.Exp, bias=0.0, scale=SCALE,
                                     accum_out=ssum)
                ssums[(hp, qb)] = ssum
        for hp in range(2):
            for qb in range(2):
                # normalize in place on DVE (ACT-Copy costs 630ns here and
                # made ACT the loop pacer; gpsimd takes 5.7us -- measured)
                nc.vector.reciprocal(rcps[hp][:, qb:qb + 1], ssums[(hp, qb)])
                nc.vector.tensor_scalar_mul(att[:, hp, qb, :],
                                            att[:, hp, qb, :],
                                            rcps[hp][:, qb:qb + 1])
        # attT[j', k, i'] = att[i', k // 3, (k % 3) * 128 + j'], k in [0,12):
        # one XBAR transpose per pair; k = hp*6 + qb*3 + w.
        attT = work.tile([128, 12, 128], F16, tag="attT", name=f"attT{ft}",
                         bufs=8)
        nc.sync.dma_start(out=attT[:, :, :], in_=att[:, 0:2, 0:2, :],
                          transpose=True)
        attTp[ft] = attT

    for ft in range(8):
        if ft >= 2:
            band_read(ft - 2)
            scores_pair(ft - 2)
        pk = ps_m.tile([128, 512], F32, tag="pm", name=f"ps_k{ft}")
        for dt in range(8):
            nc.tensor.matmul(pk, wk_sb[:, dt, ft * 128:(ft + 1) * 128],
                             catt_sb[:, dt, :], start=(dt == 0), stop=(dt == 7))
        nc.scalar.copy(kT[:, ft, :], pk)
        bd_pair(ft)
        # one v chunk per iteration: nh0 chunks first so AV pairs 0-3 are
        # unblocked right after the loop.
        jt, nh = ft % 4, ft // 4
        pv = ps_m.tile([128, 512], F32, tag="pm", name=f"ps_v{jt}_{nh}")
        for dt in range(8):
            nc.tensor.matmul(pv, catt_sb[:, dt, jt * 128:(jt + 1) * 128],
                             wv_sb[:, dt, nh * 512:(nh + 1) * 512],
                             start=(dt == 0), stop=(dt == 7))
        nc.vector.tensor_copy(val[:, jt, nh * 8:(nh + 1) * 8, :], pv)
    band_read(6)
    scores_pair(6)
    band_read(7)
    scores_pair(7)

    # ---------------- AV + incremental output projection (PE tail) --------
    # AV is val-stationary: pav[d, i] = val[j, d].T @ attT[j, i], with head
    # pairs on PE column groups 0/64.  Output is feature-major, exactly the
    # lhsT the output projection needs, so chunk ft accumulates into the 4
    # held po banks right after its pair completes.
    aoT = [persist.tile([128, 8, 128], F16, tag=f"aoT{qb}", name=f"aoT{qb}")
           for qb in range(2)]
    pav = ps_v.tile([128, 2, 128], F32, tag="pav", name="pav")
    po = [[None, None], [None, None]]
    osb = persist.tile([128, 2, DIM], F16, tag="osb", name="osb")

    for ft in range(8):
        attT = attTp[ft]
        for qb in range(2):
            for hp in range(2):
                hh = 2 * ft + hp
                for w in range(3):
                    nc.tensor.matmul(pav[hp * 64:hp * 64 + 64, qb, :],
                                     val[:, qb + w, hh, :],
                                     attT[:, hp * 6 + qb * 3 + w, :],
                                     start=(w == 0), stop=(w == 2))
        for qb in range(2):
            nc.vector.tensor_copy(aoT[qb][:, ft, :], pav[:, qb, :])
        for qb in range(2):
            for nh in range(2):
                if ft == 0:
                    po[qb][nh] = ps_m.tile([128, 512], F32, tag="pm",
                                           name=f"ps_o{qb}_{nh}")
                nc.tensor.matmul(po[qb][nh], aoT[qb][:, ft, :],
                                 wo_sb[:, ft, nh * 512:(nh + 1) * 512],
                                 start=(ft == 0), stop=(ft == 7))

    # ---------------- output writeback ----------------
    for qb in range(2):
        nc.scalar.copy(osb[:, qb, 0:512], po[qb][0])
        nc.vector.tensor_copy(osb[:, qb, 512:1024], po[qb][1])
    dst = bass.AP(out_d[:, 0:1].tensor, 0, [[DIM, 128], [128 * DIM, 2], [1, DIM]])
    nc.sync.dma_start(out=dst, in_=osb[:, :, :])


def host_prep(inputs):
    x = np.asarray(inputs["x"], dtype=np.float32)
    h = np.asarray(inputs["h"], dtype=np.float32)
    wqkv = np.asarray(inputs["Wqkv"], dtype=np.float32)
    wkr = np.asarray(inputs["Wkr"], dtype=np.float32)
    r = np.asarray(inputs["R"], dtype=np.float32)
    u = np.asarray(inputs["u"], dtype=np.float32)
    v = np.asarray(inputs["v"], dtype=np.float32)
    wout = np.asarray(inputs["Wout"], dtype=np.float32)

    # wq is stored chunk-major ([4][1024][256] flattened) so each 2-ft
    # column chunk is one contiguous 512KB block for the chunked load.
    wq_f = wqkv[:, 0:DIM].astype(np.float16)
    wq = np.ascontiguousarray(
        np.stack([wq_f[:, c * 256:(c + 1) * 256] for c in range(4)], 0)
    ).reshape(DIM, DIM)
    wk = np.ascontiguousarray(wqkv[:, DIM:2 * DIM].astype(np.float16))
    wv = np.ascontiguousarray(wqkv[:, 2 * DIM:3 * DIM].astype(np.float16))
    wo = np.ascontiguousarray(wout.astype(np.float16))

    # positional keys: only offsets s = j - i in [0, 256] are unmasked;
    # RW row for offset s is (R @ Wkr)[(s + 768) % 1024].
    rows = (np.arange(NS) + 768) % 1024
    rws = r[rows] @ wkr                        # [257, 1024] f32
    rwst = np.zeros((DIM, 258), dtype=np.float16)
    rwst[:, 0:NS] = rws.T.astype(np.float16)

    uuvv = np.stack([np.tile(u, 2), np.tile(v, 2)], axis=1)
    uuvv = np.ascontiguousarray(uuvv.astype(np.float32))

    catts = []
    for b in range(B):
        cat = np.concatenate([h[b], x[b]], axis=0)          # [512, 1024]
        catT = cat.T.astype(np.float16)                     # [dim, h|x]
        # stored column-permuted [x|h] so each half loads contiguously
        catts.append(np.ascontiguousarray(
            np.concatenate([catT[:, M:T], catT[:, 0:M]], axis=1)))

    shared = {"wq": wq, "wk": wk, "wv": wv, "wo": wo, "rwst": rwst,
              "uuvv": uuvv}
    return catts, shared


_NC_CACHE = {}


def _get_nc():
    if "nc" not in _NC_CACHE:
        _NC_CACHE["nc"] = build_kernel()
    return _NC_CACHE["nc"]


def _run(inputs, trace=False):
    catts, shared = host_prep(inputs)
    nc = _get_nc()
    in_maps = [dict(shared, catT=catts[b]) for b in range(B)]
    res = bass_utils.run_bass_kernel_spmd(
        nc, in_maps, core_ids=list(range(B)), trace=trace)
    out = np.stack([res.results[b]["out"].astype(np.float32)
                    for b in range(B)])
    return out, res


def kernel(**inputs):
    out, _ = _run(inputs, trace=False)
    return out
